# revision 3
# baseline (speedup 1.0000x reference)
"""Trainium2 Bass kernel for spatial multi-head attention (gather-attention), v2.

Computation (per agent b, H=8 heads, DK=32, K=32 neighbors, NB=16384):
    q = query @ Wq.T + bq ; k = query @ Wk.T (+bk) ; v = query @ Wv.T (+bv)
    s[b,h,k] = q[b,h,:] . k[nbr[b,k],h,:] / sqrt(DK)   (masked softmax over k)
    x[b,h,:] = sum_k p[b,h,k] v[nbr[b,k],h,:]
    out      = x @ Wo.T + bo

Algebraic simplifications (exact): bk drops out (softmax shift-invariance);
bv folds into the output bias (out = x0 @ Wo.T + (bo + Wo @ bv)).

v2 design (8 cores, data-parallel over agents, 2048 agents/core):
  - Each core redundantly projects the FULL K/V tables (PE) into ONE combined
    DRAM table kvtab[16384, 512] f16 (row = K||V, 1KB) -> HALF the gather
    descriptors of separate K/V gathers.
  - One dma_gather per 128-agent chunk (4096 rows, non-transposed,
    agent-major: row j=k*128+b -> partition b, slot k), spread across 4 SWDGE
    queues so 4 Q7 core-pairs generate descriptors concurrently (the
    dominant cost of the v1 kernel was all gathers on queue 0).
  - All elementwise work arranged for DVE perf modes: innermost dims are
    step-1 16-bit (prod mul and pv mul run 2x; reductions 4x). The
    exp-weights are pre-broadcast over DK by the Scalar engine so the pv
    multiply keeps 2x mode.
  - x = sum_k runs as a 5-step contiguous tree reduction on DVE.
  - Output projection: PE transpose of x (agents-on-partitions -> d-on-
    partitions) then two accumulating matmuls + bias matmul.
"""

import sys

if "/opt/trn_rl_repo" not in sys.path:
    sys.path.insert(0, "/opt/trn_rl_repo")

import numpy as np
from contextlib import ExitStack

H, DKD, DM = 8, 32, 256
KN = 32  # neighbors per agent
NB_FULL = 16384
NCORES = 8
SCALE = 1.0 / np.sqrt(DKD)
MASK_NEG = -30000.0  # fp16-representable; exp(scale*(s+MASK_NEG)) == 0
NQ = 4  # SWDGE queues

_PROGRAM_CACHE = {}


def _build_program(NB, NBS, mvs):
    import concourse.bacc as bacc
    import concourse.tile as tile
    import concourse.mybir as mybir
    from concourse.tile_rust import add_dep_helper
    from concourse.library_config import mlp as mlp_lib

    f16 = mybir.dt.float16
    f32 = mybir.dt.float32
    i16 = mybir.dt.int16
    Act = mybir.ActivationFunctionType
    X = mybir.AxisListType.X
    ADD = mybir.AluOpType.add

    CH = NBS // 128        # chunks of 128 agents
    NBT = NB // 128        # table row-tiles
    WG = 8                 # table tiles batched per DRAM write
    RW = 2 * DM            # combined row width (K||V)

    nc = bacc.Bacc("TRN2", target_bir_lowering=False, debug=False,
                   num_swdge_queues=NQ)

    qT = nc.dram_tensor("qT", [DM, NB], f16, kind="ExternalInput").ap()
    qTs = nc.dram_tensor("qTs", [DM, NBS], f16, kind="ExternalInput").ap()
    WqT = nc.dram_tensor("WqT", [DM, DM], f16, kind="ExternalInput").ap()
    WkT = nc.dram_tensor("WkT", [DM, DM], f16, kind="ExternalInput").ap()
    WvT = nc.dram_tensor("WvT", [DM, DM], f16, kind="ExternalInput").ap()
    WoA = nc.dram_tensor("WoA", [DM, DM], f16, kind="ExternalInput").ap()
    bqr = nc.dram_tensor("bqr", [1, DM], f16, kind="ExternalInput").ap()
    boeff = nc.dram_tensor("boeff", [1, DM], f16, kind="ExternalInput").ap()
    ones1 = nc.dram_tensor("ones1", [1, 128], f16, kind="ExternalInput").ap()
    ident = nc.dram_tensor("ident", [128, 128], f16, kind="ExternalInput").ap()
    idxKV = nc.dram_tensor("idxKV", [128, CH * 256], i16, kind="ExternalInput").ap()
    maskM = nc.dram_tensor("maskM", [128, CH * KN], f16, kind="ExternalInput").ap()
    outp = nc.dram_tensor("out", [NBS, DM], f16, kind="ExternalOutput").ap()

    kvtab = nc.dram_tensor("kvtab", [NB, RW], f16).ap()

    with tile.TileContext(nc) as tc:
        with ExitStack() as ctx:
            libload = nc.gpsimd.load_library(mlp_lib)
            consts = ctx.enter_context(tc.tile_pool(name="consts", bufs=1))

            def load_const(name, ap, shape, dtype, rearr=None):
                t = consts.tile(shape, dtype, tag=name)
                src = ap if rearr is None else ap.rearrange(rearr, p=128)
                nc.sync.dma_start(t[:], src)
                return t

            wq_sb = load_const("wq", WqT, [128, 2, DM], f16, "(c p) d -> p c d")
            wk_sb = load_const("wk", WkT, [128, 2, DM], f16, "(c p) d -> p c d")
            wv_sb = load_const("wv", WvT, [128, 2, DM], f16, "(c p) d -> p c d")
            wo_sb = load_const("wo", WoA, [128, 2, DM], f16, "(c p) d -> p c d")
            bq_sb = load_const("bq", bqr, [1, DM], f16)
            bo_sb = load_const("bo", boeff, [1, DM], f16)
            on1_sb = load_const("on1", ones1, [1, 128], f16)
            id_sb = load_const("id", ident, [128, 128], f16)
            ix_sb = load_const("ix", idxKV, [128, CH * 256], i16)
            msk_sb = load_const("msk", maskM, [128, CH * KN], f16)

            qnp = ctx.enter_context(tc.tile_pool(name="qnp", bufs=1))
            qn_all = qnp.tile([128, CH, DM], f16, tag="qn")

            # ---------------- Phase A: projections ----------------
            kvwr = []
            with ExitStack() as actx:
                qpool = actx.enter_context(tc.tile_pool(name="qtp", bufs=1))
                qt_sb = qpool.tile([128, 2, NB], f16, tag="qt")
                QSP = NB // 4
                for qs_i in range(4):
                    nc.sync.dma_start(
                        qt_sb[:, :, qs_i * QSP : (qs_i + 1) * QSP],
                        qT[:, qs_i * QSP : (qs_i + 1) * QSP].rearrange(
                            "(c p) b -> p c b", p=128
                        ),
                    )
                qs_sb = qpool.tile([128, 2, NBS], f16, tag="qs")
                nc.sync.dma_start(qs_sb[:], qTs.rearrange("(c p) b -> p c b", p=128))

                astg = actx.enter_context(tc.tile_pool(name="astg", bufs=3))

                # K/V tables -> combined kvtab rows (K cols 0:256, V 256:512)
                GB = 4  # row-tiles batched per PSUM tile / staging copy
                taps = ExitStack()
                aps = taps.enter_context(
                    tc.tile_pool(name="aps", bufs=2, space="PSUM")
                )
                for g in range(NBT // WG):
                    kstg = astg.tile([128, WG, DM], f16, tag="kstg")
                    vstg = astg.tile([128, WG, DM], f16, tag="vstg")
                    for jb in range(WG // GB):
                        kp = aps.tile([128, GB, DM], f32, tag="kp")
                        vp = aps.tile([128, GB, DM], f32, tag="vp")
                        for j in range(GB):
                            bt = g * WG + jb * GB + j
                            for ih in range(2):
                                # same stationary qt slice for K and V
                                nc.tensor.matmul(
                                    kp[:, j, :],
                                    lhsT=qt_sb[:, ih, bt * 128 : bt * 128 + 128],
                                    rhs=wk_sb[:, ih, :],
                                    start=(ih == 0),
                                    stop=(ih == 1),
                                )
                                nc.tensor.matmul(
                                    vp[:, j, :],
                                    lhsT=qt_sb[:, ih, bt * 128 : bt * 128 + 128],
                                    rhs=wv_sb[:, ih, :],
                                    start=(ih == 0),
                                    stop=(ih == 1),
                                )
                        sl = slice(jb * GB, jb * GB + GB)
                        nc.scalar.copy(kstg[:, sl, :], kp[:])
                        nc.vector.tensor_copy(vstg[:, sl, :], vp[:])
                    rows = 128 * WG
                    kvwr.append(
                        nc.sync.dma_start(
                            kvtab[g * rows : (g + 1) * rows, 0:DM].rearrange(
                                "(j p) d -> p j d", p=128
                            ),
                            kstg[:],
                        )
                    )
                    kvwr.append(
                        nc.sync.dma_start(
                            kvtab[g * rows : (g + 1) * rows, DM:RW].rearrange(
                                "(j p) d -> p j d", p=128
                            ),
                            vstg[:],
                        )
                    )

                taps.close()
                # per-chunk q rows (agents on partitions): qn = qTs.T @ Wq.T+bq
                # after the tables so gathers start while q-proj still runs
                qaps = actx.enter_context(
                    tc.tile_pool(name="qaps", bufs=2, space="PSUM")
                )
                for ch in range(CH):
                    qp = qaps.tile([128, DM], f32, tag="qproj")
                    for ih in range(2):
                        nc.tensor.matmul(
                            qp[:],
                            lhsT=qs_sb[:, ih, ch * 128 : ch * 128 + 128],
                            rhs=wq_sb[:, ih, :],
                            start=(ih == 0),
                            stop=False,
                            skip_group_check=(ih == 1),
                        )
                    nc.tensor.matmul(
                        qp[:], lhsT=on1_sb[:], rhs=bq_sb[:],
                        start=False, stop=True, skip_group_check=True,
                    )
                    nc.scalar.copy(qn_all[:, ch, :], qp[:])

            # ---------------- Phase B: gather + attention ----------------
            KVS = max(mvs)
            kv_bytes = KVS * RW * 2
            kv_bufs = 5 if kv_bytes * 5 <= 140 * 1024 else 4
            kvp = ctx.enter_context(tc.tile_pool(name="kvp", bufs=kv_bufs))
            prp = ctx.enter_context(tc.tile_pool(name="prp", bufs=2))
            sfx = ctx.enter_context(tc.tile_pool(name="sfx", bufs=3))
            pst = ctx.enter_context(tc.tile_pool(name="pst", bufs=2, space="PSUM"))
            pso = ctx.enter_context(tc.tile_pool(name="pso", bufs=2, space="PSUM"))

            for ch in range(CH):
                mv = mvs[ch]
                kv = kvp.tile([128, KVS, RW], f16, tag="kv")
                # four quarter-gathers, one per SWDGE queue: descriptor
                # generation runs on all four Q7 core-pairs concurrently
                qs = mv // 4
                cuts = [0, qs, 2 * qs, 3 * qs, mv]
                for gh, (lo, hi) in enumerate(zip(cuts[:-1], cuts[1:])):
                    gi = nc.gpsimd.dma_gather(
                        kv[:, lo:hi, :],
                        kvtab,
                        ix_sb[:, ch * 256 + 8 * lo : ch * 256 + 8 * hi],
                        num_idxs=128 * (hi - lo),
                        num_idxs_reg=128 * (hi - lo),
                        elem_size=RW,
                        transpose=False,
                        single_packet=False,
                        queue_num=gh % NQ,
                    )
                    add_dep_helper(gi.ins, libload.ins, sync=True)
                    for w in kvwr:
                        add_dep_helper(gi.ins, w.ins, sync=True)

                # All compute below touches only the mv gathered slots; the
                # kv tail (mv:KN) is never read, so no masked-tail hazards.
                # prod/pv share one [128, 8192] f16 tile (disjoint lifetimes)
                big = prp.tile([128, KN * DM], f16, tag="big")
                prod = big[:].rearrange("p (k h d) -> p k h d", k=KN, h=H)
                # scores products, flat 256-elem runs
                nc.vector.tensor_mul(
                    big[:].rearrange("p (k d) -> p k d", k=KN)[:, 0:mv, :],
                    kv[:, 0:mv, 0:DM],
                    qn_all[:, ch : ch + 1, :].broadcast_to([128, mv, DM]),
                )
                # score sums over dd: in-place contiguous tree (stays in 2x)
                pr = prod[:, 0:mv]
                nc.vector.tensor_add(
                    pr[:, :, :, 0:16], pr[:, :, :, 0:16], pr[:, :, :, 16:32]
                )
                nc.vector.tensor_add(
                    pr[:, :, :, 0:8], pr[:, :, :, 0:8], pr[:, :, :, 8:16]
                )
                nc.vector.tensor_add(
                    pr[:, :, :, 0:4], pr[:, :, :, 0:4], pr[:, :, :, 4:8]
                )
                nc.vector.tensor_add(
                    pr[:, :, :, 0:2], pr[:, :, :, 0:2], pr[:, :, :, 2:4]
                )
                sc = sfx.tile([128, KN, H], f16, tag="sc")
                scv = sc[:, 0:mv].rearrange("p k (h u) -> p k h u", u=1)
                nc.vector.tensor_add(scv, pr[:, :, :, 0:1], pr[:, :, :, 1:2])
                sm = sfx.tile([128, KN, H], f16, tag="sm")
                nc.vector.tensor_add(
                    sm[:, 0:mv],
                    sc[:, 0:mv],
                    msk_sb[:, ch * KN : ch * KN + mv]
                    .rearrange("p (k u) -> p k u", u=1)
                    .broadcast_to([128, mv, H]),
                )
                ex = sfx.tile([128, KN, H], f16, tag="ex")
                nc.scalar.activation(
                    ex[:, 0:mv], sm[:, 0:mv], Act.Exp, scale=float(SCALE)
                )
                z = sfx.tile([128, H], f32, tag="z")
                nc.vector.tensor_reduce(
                    z[:],
                    ex[:, 0:mv].rearrange("p k h -> p h k"),
                    axis=X,
                    op=ADD,
                )
                rz = sfx.tile([128, H], f32, tag="rz")
                nc.vector.reciprocal(rz[:], z[:])
                rz16 = sfx.tile([128, H], f16, tag="rz16")
                nc.scalar.copy(rz16[:], rz[:])

                # expand UNNORMALIZED exp-weights over DK on the Scalar engine
                # (normalization applied to x at the end); into kv's K-half
                # (dead after the score products)
                pvw = kv[:, 0:mv, 0:DM].rearrange("p k (h d) -> p k h d", h=H)
                nc.scalar.activation(
                    pvw,
                    ex[:, 0:mv]
                    .rearrange("p k (h u) -> p k h u", u=1)
                    .broadcast_to([128, mv, H, DKD]),
                    Act.Identity,
                    scale=1.0,
                )

                # pv = Vg * pvw, into the big tile (flat 256-elem runs)
                nc.vector.tensor_mul(
                    big[:].rearrange("p (k d) -> p k d", k=KN)[:, 0:mv, :],
                    kv[:, 0:mv, DM:RW],
                    kv[:, 0:mv, 0:DM],
                )

                # x = sum_(mv slots) pv : ragged tree, scratch in kv's K-half
                pvm = big[:].rearrange("p (m d) -> p m d", d=DM)
                xt = kv[:, :, 0:DM]
                h1 = mv // 2
                nc.vector.tensor_add(
                    xt[:, 0:h1, :], pvm[:, 0:h1, :], pvm[:, h1 : 2 * h1, :]
                )
                if mv % 2:
                    nc.vector.tensor_copy(
                        xt[:, h1 : h1 + 1, :], pvm[:, 2 * h1 : mv, :]
                    )
                n = h1 + (mv % 2)
                while n > 2:
                    hh = n // 2
                    nc.vector.tensor_add(
                        xt[:, 0:hh, :], xt[:, 0:hh, :], xt[:, hh : 2 * hh, :]
                    )
                    if n % 2:
                        nc.vector.tensor_add(
                            xt[:, 0:1, :], xt[:, 0:1, :], xt[:, n - 1 : n, :]
                        )
                    n = hh
                x16 = sfx.tile([128, DM], f16, tag="x16")
                if n == 2:
                    nc.vector.tensor_add(x16[:], xt[:, 0, :], xt[:, 1, :])
                else:
                    nc.vector.tensor_copy(x16[:], xt[:, 0, :])
                # normalize by the softmax denominators (per agent, head)
                nc.vector.tensor_mul(
                    x16[:].rearrange("p (h d) -> p h d", h=H),
                    x16[:].rearrange("p (h d) -> p h d", h=H),
                    rz16[:]
                    .rearrange("p (h u) -> p h u", u=1)
                    .broadcast_to([128, H, DKD]),
                )

                # transpose x (agents-on-partitions -> d-on-partitions)
                xTp = pst.tile([128, 2, 128], f16, tag="xTp")
                for c in range(2):
                    nc.tensor.transpose(
                        xTp[:, c, :], x16[:, c * 128 : c * 128 + 128], id_sb[:]
                    )
                xT16 = sfx.tile([128, 2, 128], f16, tag="xT16")
                nc.scalar.copy(xT16[:], xTp[:])

                # output projection + bias
                op = pso.tile([128, DM], f32, tag="op")
                for c in range(2):
                    nc.tensor.matmul(
                        op[:],
                        lhsT=xT16[:, c, :],
                        rhs=wo_sb[:, c, :],
                        start=(c == 0),
                        stop=False,
                        skip_group_check=(c == 1),
                    )
                nc.tensor.matmul(
                    op[:], lhsT=on1_sb[:], rhs=bo_sb[:],
                    start=False, stop=True, skip_group_check=True,
                )
                ou = sfx.tile([128, DM], f16, tag="ou")
                nc.scalar.copy(ou[:], op[:])
                nc.sync.dma_start(outp[ch * 128 : ch * 128 + 128, :], ou[:])

    nc.compile()
    return nc


def _host_prep(query_, spatial_neighbors, mask, Wq, bq, Wk, bk, Wv, bv, Wo, bo,
               NB, NBS, ncores):
    """Pure-layout host prep: transposes, fp16 casts, index/mask relayout."""
    CH = NBS // 128
    f16 = np.float16

    q32 = np.asarray(query_, np.float32)
    qT16 = np.ascontiguousarray(q32.T).astype(f16)
    WqT16 = np.ascontiguousarray(np.asarray(Wq, np.float32).T).astype(f16)
    WkT16 = np.ascontiguousarray(np.asarray(Wk, np.float32).T).astype(f16)
    WvT16 = np.ascontiguousarray(np.asarray(Wv, np.float32).T).astype(f16)
    WoA16 = np.ascontiguousarray(np.asarray(Wo, np.float32).T).astype(f16)
    bq16 = np.asarray(bq, np.float32).astype(f16).reshape(1, DM)
    boe = (np.asarray(bo, np.float64)
           + np.asarray(Wo, np.float64) @ np.asarray(bv, np.float64))
    boe16 = boe.astype(np.float32).astype(f16).reshape(1, DM)
    ones1 = np.ones((1, 128), f16)
    ident = np.eye(128, dtype=f16)

    nbr = np.asarray(spatial_neighbors, np.int64)
    msk = np.asarray(mask, np.int32).reshape(NB, KN) != 0

    def wrap16(flat):
        # flat index i at [i%16, i//16], replicated 8x for the 8 Q7 cores
        return np.tile(flat.reshape(-1, 16).T, (8, 1)).astype(np.int16)

    # Pack each agent's unmasked neighbors into the low k-slots so the
    # common masked tail of each chunk can be trimmed from the gather
    # (dma_gather ignores trailing negative indices). Chunks < 4 stay
    # full: their kv buffers see first use and trimmed slots would read
    # uninitialized SBUF (NaN risk in the masked lanes).
    order = np.argsort(~msk, axis=1, kind="stable")
    nbr_p = np.take_along_axis(nbr, order, axis=1)
    msk_p = np.take_along_axis(msk, order, axis=1)
    vcnt = msk.sum(1)

    per_core = []
    for c in range(ncores):
        base = c * NBS
        sl = slice(base, base + NBS)
        qTs16 = np.ascontiguousarray(q32[sl].T).astype(f16)

        cols = []
        for ch in range(CH):
            a = slice(base + ch * 128, base + ch * 128 + 128)
            fl = np.where(msk_p[a], nbr_p[a], 0).astype(np.int64)  # [128, KN]
            cols.append(wrap16(fl.T.reshape(-1)))  # j = k*128 + b
        iKV = np.concatenate(cols, axis=1)

        mM = np.where(msk_p[sl], 0.0, MASK_NEG).astype(f16)
        # [NBS, KN] -> [128, CH*KN] : agent ch*128+p at [p, ch*32+k]
        mM = np.ascontiguousarray(
            mM.reshape(CH, 128, KN).transpose(1, 0, 2).reshape(128, CH * KN)
        )

        per_core.append(
            dict(
                qT=qT16, qTs=qTs16, WqT=WqT16, WkT=WkT16, WvT=WvT16, WoA=WoA16,
                bqr=bq16, boeff=boe16, ones1=ones1, ident=ident,
                idxKV=iKV, maskM=mM,
            )
        )
    # static per-chunk gather counts: max valid slots across all cores;
    # first 4 chunks stay full (first use of each kv ring buffer)
    mvs = []
    for ch in range(CH):
        m = max(
            int(vcnt[c * NBS + ch * 128 : c * NBS + ch * 128 + 128].max())
            for c in range(ncores)
        )
        mvs.append(min(m, KN))
    return per_core, tuple(mvs)


def kernel(**inputs):
    NB, NBS = NB_FULL, NB_FULL // NCORES
    in_maps, mvs = _host_prep(NB=NB, NBS=NBS, ncores=NCORES, **inputs)
    key = (NB, NBS, mvs)
    if key not in _PROGRAM_CACHE:
        _PROGRAM_CACHE[key] = _build_program(NB, NBS, mvs)
    nc = _PROGRAM_CACHE[key]

    from concourse.bass_utils import run_bass_kernel_spmd

    res = run_bass_kernel_spmd(nc, in_maps, list(range(NCORES)))
    out = np.concatenate([res.results[c]["out"] for c in range(NCORES)], axis=0)
    return out.reshape(NB, 1, DM).astype(np.float32)
